# revision 1
# baseline (speedup 1.0000x reference)
"""Multi-head attention (B=2, S=2048, D=1024, H=16) on 8 TRN2 NeuronCores.

Sharding: tensor parallel over heads (2 heads/core) for QKV projection +
attention, then an AllToAll of the context (channel-shard -> row-shard),
then row-parallel output projection. Inputs arrive full; sharding happens
host-side in `kernel()`.

Matmuls run in bf16 (fp32r leaves the PE clock-gated cold and costs 1.5
cyc/row; bf16 is 1 cyc/row, warms HAM, and gets fast weight loads).
Softmax statistics stay fp32 in PSUM; 1/sum is computed as exp(-ln(s)) on
the Scalar engine so the Vector engine never blocks the PE pipeline.

The attention q-range is strided across cores so each of the two AllToAll
halves carries a fully-populated buffer, letting collective #1 and the
first half of the output projection overlap the second attention pass.

Self-contained: shapes hardcoded, no sibling imports.
"""

import numpy as np

B, S, D, H = 2, 2048, 1024, 16
NCORES = 8
CH = D // NCORES          # 128 channels (2 heads) per core
HD = D // H               # 64
ROWS = B * S              # 4096
RPC = ROWS // NCORES      # 512 rows per core for the output projection
KO = D // 128             # 8 contraction chunks of 128
QCH = 1024                # q-chunk processed per attention pass
NQ = S // QCH             # 2 passes
KB = S // 128             # 16 key blocks
RH = RPC // NQ            # 256 rows per core per A2A half
SCALE = 1.0 / 32.0        # 1/sqrt(D)

_CACHE = {}


def _build():
    import concourse.mybir as mybir
    import concourse.tile as tile
    from concourse import bacc
    from concourse.masks import make_identity

    BF16 = mybir.dt.bfloat16
    F32 = mybir.dt.float32
    AF = mybir.ActivationFunctionType

    nc = bacc.Bacc("TRN2", target_bir_lowering=False, debug=False, num_devices=NCORES)
    xT = nc.dram_tensor("xT", [D, ROWS], BF16, kind="ExternalInput")
    # weights arrive host-pre-tiled as [128, KO, out] so DMAs are contiguous
    wq = nc.dram_tensor("wq", [128, KO, CH], BF16, kind="ExternalInput")
    wk = nc.dram_tensor("wk", [128, KO, CH], BF16, kind="ExternalInput")
    wv = nc.dram_tensor("wv", [128, KO, CH], BF16, kind="ExternalInput")
    wo = nc.dram_tensor("wo", [128, KO, D], BF16, kind="ExternalInput")
    out = nc.dram_tensor("out", [RPC, D], F32, kind="ExternalOutput")

    with tile.TileContext(nc) as tc:
        with (
            tc.tile_pool(name="const", bufs=1) as cpool,
            tc.tile_pool(name="qkv", bufs=16) as qkvp,
            tc.tile_pool(name="vt", bufs=3) as vtp,
            tc.tile_pool(name="vtr", bufs=8) as vtrp,
            tc.tile_pool(name="xt", bufs=6) as xtp,
            tc.tile_pool(name="exp", bufs=6) as expp,
            tc.tile_pool(name="bc", bufs=2) as bcp,
            tc.tile_pool(name="cs", bufs=2) as csp,
            tc.tile_pool(name="ph2", bufs=2) as ph2p,
            tc.tile_pool(name="osb", bufs=2) as osbp,
            tc.tile_pool(name="ps", bufs=2, space="PSUM") as ps,
            tc.tile_pool(name="dram", bufs=1, space="DRAM") as dram,
        ):
            w_tiles = {}
            for name, t in (("wq", wq), ("wk", wk), ("wv", wv)):
                wt = cpool.tile([128, KO, CH], BF16, tag=name)
                nc.sync.dma_start(wt[:], t[:])
                w_tiles[name] = wt
            ident = cpool.tile([128, 128], BF16, tag="ident")
            make_identity(nc, ident[:])

            a2a_in = [dram.tile([NCORES, CH, RH], BF16, name=f"a2a_in{p}") for p in range(NQ)]
            a2a_out = [dram.tile([NCORES, CH, RH], BF16, name=f"a2a_out{p}") for p in range(NQ)]

            xT_r = xT.ap().rearrange("(ko p) n -> p ko n", p=128)
            NRB = S // 512  # rowblocks per batch

            qts = {0: [None] * NQ, 1: [None] * NQ}
            kts = {0: [None] * NRB, 1: [None] * NRB}
            vrs = {0: [None] * NRB, 1: [None] * NRB}

            def proj_rowblock(b, rb):
                """project one 512-row block; V transposed into row-major
                [krows, ch] blocks with a fused ones column per head."""
                r = b * NRB + rb
                xt = xtp.tile([128, KO, 512], BF16, tag="xt")
                nc.sync.dma_start(xt[:], xT_r[:, :, r * 512:(r + 1) * 512])
                # q is stored per head, padded with a zeroed half, so the
                # scores matmul can contract over the full 128 partitions --
                # same cycle count, but the PE array streams full-width (keeps
                # the HAM clock gate warm)
                if rb % 2 == 0:
                    qp = [
                        qkvp.tile([128, QCH], BF16, tag="qt",
                                  name=f"qt{b}_{rb // 2}_{h}")
                        for h in range(2)
                    ]
                    nc.vector.memset(qp[0][64:128, :], 0.0)
                    nc.vector.memset(qp[1][0:64, :], 0.0)
                    qts[b][rb // 2] = qp
                qt = qts[b][rb // 2]
                qoff = (rb % 2) * 512
                kt = qkvp.tile([128, 512], BF16, tag="kt", name=f"kt{b}_{rb}")
                vt = vtp.tile([128, 512], BF16, tag="vt")
                for wname, dst in (("wq", None), ("wk", kt), ("wv", vt)):
                    pj = ps.tile([128, 512], F32, tag="sc")
                    for ko in range(KO):
                        nc.tensor.matmul(
                            pj[:], w_tiles[wname][:, ko, :], xt[:, ko, :],
                            start=(ko == 0), stop=(ko == KO - 1),
                        )
                    if wname == "wq":
                        # each head's q lives on the partitions matching its
                        # own channels in kt; the other half stays zero
                        nc.vector.tensor_copy(
                            qt[0][0:64, qoff:qoff + 512], pj[0:64, :])
                        nc.vector.tensor_copy(
                            qt[1][64:128, qoff:qoff + 512], pj[64:128, :])
                    else:
                        nc.vector.tensor_copy(dst[:], pj[:])
                # vr: per head a full-128 lhsT block [V_h | 1 | zeros]
                vr = vtrp.tile([128, 4, 256], BF16, tag="vtr", name=f"vr{b}_{rb}")
                nc.vector.memset(vr[:], 0.0)
                nc.vector.memset(vr[:, :, 64:65], 1.0)
                nc.vector.memset(vr[:, :, 192:193], 1.0)
                for j in range(4):
                    tp = ps.tile([128, 128], BF16, tag="sc", name=f"tp{b}_{rb}_{j}")
                    nc.tensor.transpose(tp[:], vt[:, j * 128:(j + 1) * 128], ident[:])
                    nc.vector.tensor_copy(vr[:, j, 0:64], tp[:, 0:64])
                    nc.vector.tensor_copy(vr[:, j, 128:192], tp[:, 64:128])
                kts[b][rb], vrs[b][rb] = kt, vr

            def attn_chunk(b, p, ctx_ps, kb_range):
                for kb in kb_range:
                    krb, kj = kb // 4, kb % 4
                    scs = [
                        ps.tile([128, QCH], F32, tag="sc", name=f"sc_{b}_{p}_{kb}_{h}")
                        for h in range(2)
                    ]
                    # full-partition contraction: the upper half of each
                    # padded q tile is zero, so the other head's k rows
                    # contribute nothing
                    for h in range(2):
                        for n in range(QCH // 512):
                            nc.tensor.matmul(
                                scs[h][:, n * 512:(n + 1) * 512],
                                kts[b][krb][:, kj * 128:(kj + 1) * 128],
                                qts[b][p][h][:, n * 512:(n + 1) * 512],
                                start=True, stop=True,
                            )
                    ex = []
                    for h in range(2):
                        e = expp.tile([128, QCH], BF16, tag="exp")
                        nc.scalar.activation(e[:], scs[h][:], AF.Exp, scale=SCALE)
                        ex.append(e)
                    for h in range(2):
                        for n in range(QCH // 512):
                            nc.tensor.matmul(
                                ctx_ps[h][:, n * 512:(n + 1) * 512],
                                vrs[b][krb][:, kj, h * 128:(h + 1) * 128],
                                ex[h][:, n * 512:(n + 1) * 512],
                                start=(kb == 0), stop=(kb == KB - 1),
                            )

            def free_ctx(b, p, ctx_ps):
                # fast DVE copies release the ctx psum slots for the next pass
                cfs = []
                for h in range(2):
                    cf = csp.tile([65, QCH], F32, tag="cf", name=f"cf_{b}_{p}_{h}")
                    nc.vector.tensor_copy(cf[:], ctx_ps[h][0:65, :])
                    cfs.append(cf)
                return cfs

            def ship(b, p, cfs, last=False):
                # normalize from SBUF and scatter into the A2A buffer; emitted
                # after any later input-prefetch DMAs so those aren't queued
                # behind these writes
                for h in range(2):
                    cf = cfs[h]
                    bc = bcp.tile([64, QCH], F32, tag="bc")
                    if last:
                        # tail: ACT's ln+exp reciprocal is ~3x faster than
                        # DVE's exact reciprocal and ACT is idle by now
                        lt = bcp.tile([1, QCH], F32, tag="lt")
                        nc.scalar.activation(lt[:], cf[64:65, :], AF.Ln)
                        nc.scalar.activation(bc[0:1, :], lt[:], AF.Exp, scale=-1.0)
                    else:
                        nc.vector.reciprocal(bc[0:1, :], cf[64:65, :])
                    nc.gpsimd.partition_broadcast(bc[:], bc[0:1, :], channels=64)
                    cs = csp.tile([64, QCH], BF16, tag="cs")
                    nc.vector.tensor_mul(cs[:], cf[0:64, :], bc[:])
                    # q within the pass decomposes as (v, j, i) -> dst core
                    # 4b+j, local row v*128+i
                    nc.sync.dma_start(
                        a2a_in[p][4 * b:4 * b + 4, h * 64:(h + 1) * 64, :]
                        .rearrange("j c (v i) -> c v j i", i=128),
                        cs[:].rearrange("c (v j i) -> c v j i", v=2, j=4),
                    )

            def phase2_half(p, wo_t):
                ctxg = ph2p.tile([128, KO, RH], BF16, tag="ctxg", name=f"ctxg{p}")
                nc.sync.dma_start(ctxg[:], a2a_out[p][:].rearrange("j q r -> q j r"))
                for rb in range(RH // 128):
                    for nh in range(D // 512):
                        pj = ps.tile(
                            [128, 512], F32,
                            tag=("cx" if (rb + nh) % 2 else "sc"),
                            name=f"p2_{p}_{rb}_{nh}",
                        )
                        for j in range(KO):
                            nc.tensor.matmul(
                                pj[:],
                                ctxg[:, j, rb * 128:(rb + 1) * 128],
                                wo_t[:, j, nh * 512:(nh + 1) * 512],
                                start=(j == 0), stop=(j == KO - 1),
                            )
                        ob = osbp.tile([128, 512], F32, tag="osb")
                        nc.vector.tensor_copy(ob[:], pj[:])
                        nc.sync.dma_start(
                            out.ap()[p * RH + rb * 128:p * RH + (rb + 1) * 128,
                                     nh * 512:(nh + 1) * 512],
                            ob[:],
                        )

            def ctx_alloc(b, p):
                return [
                    ps.tile([128, QCH], F32, tag="cx", name=f"ctx_{b}_{p}_{h}")
                    for h in range(2)
                ]

            # pass p=0 pipelines the projection inside the attention kblk
            # loop (ScalarE starts ~50us earlier); proj psum shares the "sc"
            # FIFO with scores in emission order
            proj_rowblock(0, 0)
            proj_rowblock(0, 1)
            ctx00 = ctx_alloc(0, 0)
            attn_chunk(0, 0, ctx00, range(0, 4))
            proj_rowblock(0, 2)
            attn_chunk(0, 0, ctx00, range(4, 8))
            proj_rowblock(0, 3)
            attn_chunk(0, 0, ctx00, range(8, 16))
            cfs00 = free_ctx(0, 0, ctx00)
            proj_rowblock(1, 0)
            proj_rowblock(1, 1)
            ctx10 = ctx_alloc(1, 0)
            attn_chunk(1, 0, ctx10, range(0, 4))
            proj_rowblock(1, 2)
            attn_chunk(1, 0, ctx10, range(4, 8))
            proj_rowblock(1, 3)
            # ship after b1's proj so its slow reciprocal doesn't sit ahead of
            # the proj psum-freeing copies in the in-order DVE queue
            ship(0, 0, cfs00)
            wo_t = cpool.tile([128, KO, D], BF16, tag="wo")
            nc.sync.dma_start(wo_t[:], wo[:])
            attn_chunk(1, 0, ctx10, range(8, 16))
            cfs10 = free_ctx(1, 0, ctx10)
            ship(1, 0, cfs10)
            nc.gpsimd.collective_compute(
                "AllToAll", mybir.AluOpType.bypass,
                replica_groups=[list(range(NCORES))],
                ins=[a2a_in[0].opt()], outs=[a2a_out[0].opt()],
            )
            for b in range(B):
                ctx = ctx_alloc(b, 1)
                attn_chunk(b, 1, ctx, range(KB))
                cfs = free_ctx(b, 1, ctx)
                ship(b, 1, cfs, last=(b == 1))
            # emitted before the collective: Tile orders post-collective work
            # after it, so half 0 (whose data arrived with collective #0)
            # must precede to fill the skew window while A2A#1 completes
            phase2_half(0, wo_t)
            nc.gpsimd.collective_compute(
                "AllToAll", mybir.AluOpType.bypass,
                replica_groups=[list(range(NCORES))],
                ins=[a2a_in[1].opt()], outs=[a2a_out[1].opt()],
            )
            phase2_half(1, wo_t)
    nc.compile()
    return nc


def _numpy_reference(tensor_in, attention_mask, Wq, Wk, Wv, Wo):
    """Fallback for a non-zero mask (never hit with the spec's zero mask)."""
    x = tensor_in.astype(np.float64)
    q = (x @ Wq.T.astype(np.float64)).reshape(B, S, H, HD).transpose(0, 2, 1, 3)
    k = (x @ Wk.T.astype(np.float64)).reshape(B, S, H, HD).transpose(0, 2, 1, 3)
    v = (x @ Wv.T.astype(np.float64)).reshape(B, S, H, HD).transpose(0, 2, 1, 3)
    scores = np.einsum("bhqd,bhkd->bhqk", q, k) + attention_mask.astype(np.float64)
    scores = scores / np.sqrt(D)
    scores -= scores.max(axis=-1, keepdims=True)
    w = np.exp(scores)
    w /= w.sum(axis=-1, keepdims=True)
    ctx = np.einsum("bhqk,bhkd->bhqd", w, v).transpose(0, 2, 1, 3).reshape(B, S, D)
    return (ctx @ Wo.T.astype(np.float64)).astype(np.float32)


def _pretile(wT: np.ndarray) -> np.ndarray:
    """[D, M] -> [128, KO, M] with row d = ko*128 + p."""
    m = wT.shape[1]
    return np.ascontiguousarray(wT.reshape(KO, 128, m).transpose(1, 0, 2))


def _row_map() -> np.ndarray:
    """global row index handled by (core c, local row lr)."""
    m = np.empty((NCORES, RPC), dtype=np.int64)
    for c in range(NCORES):
        bb, jj = c // 4, c % 4
        for p in range(NQ):
            for rb in range(RH // 128):
                u = 2 * p + rb
                g = bb * S + jj * 128 + 512 * u
                lr = p * RH + rb * 128
                m[c, lr:lr + 128] = np.arange(g, g + 128)
    return m


def _run(inputs, trace=False):
    import ml_dtypes
    from concourse.bass_utils import run_bass_kernel_spmd

    bf16 = ml_dtypes.bfloat16
    tensor_in = np.asarray(inputs["tensor_in"], dtype=np.float32)
    Wq = np.asarray(inputs["Wq"], dtype=np.float32)
    Wk = np.asarray(inputs["Wk"], dtype=np.float32)
    Wv = np.asarray(inputs["Wv"], dtype=np.float32)
    Wo = np.asarray(inputs["Wo"], dtype=np.float32)

    xT = np.ascontiguousarray(tensor_in.reshape(ROWS, D).T).astype(bf16)
    wqT = Wq.T.astype(bf16)
    wkT = Wk.T.astype(bf16)
    wvT = Wv.T.astype(bf16)
    wo_p = _pretile(Wo.T.astype(bf16))

    in_maps = []
    for c in range(NCORES):
        sl = slice(c * CH, (c + 1) * CH)
        in_maps.append({
            "xT": xT,
            "wq": _pretile(wqT[:, sl]),
            "wk": _pretile(wkT[:, sl]),
            "wv": _pretile(wvT[:, sl]),
            "wo": wo_p,
        })

    if "nc" not in _CACHE:
        _CACHE["nc"] = _build()
    res = run_bass_kernel_spmd(
        _CACHE["nc"], in_maps, core_ids=list(range(NCORES)), trace=trace
    )
    rm = _CACHE.setdefault("rm", _row_map())
    full = np.empty((ROWS, D), dtype=np.float32)
    for c in range(NCORES):
        full[rm[c]] = res.results[c]["out"]
    return full.reshape(B, S, D), res


def kernel(**inputs) -> np.ndarray:
    mask = np.asarray(inputs["attention_mask"])
    if mask.any():
        return _numpy_reference(
            np.asarray(inputs["tensor_in"]), mask,
            np.asarray(inputs["Wq"]), np.asarray(inputs["Wk"]),
            np.asarray(inputs["Wv"]), np.asarray(inputs["Wo"]),
        )
    out, _ = _run(inputs, trace=False)
    return out



# revision 2
# speedup vs baseline: 1.0409x; 1.0409x over previous
"""Multi-head attention (B=2, S=2048, D=1024, H=16) on 8 TRN2 NeuronCores, v2.

Sharding: tensor parallel over heads (2 heads/core). Per core:
  - QKV projection of the full 4096 rows onto its 128 channels.
  - Scores with K=64 contraction per head (tile_position picks the
    64-partition quadrant; no zero padding).
  - Context in [q, ch] orientation: lhsT = exp-scores [keys, q-block],
    rhs = [V | 1] rows -> full 128x128 PE utilization and the ones
    column lands the softmax denominator in the same PSUM tile, so the
    normalize is a per-partition tensor_scalar multiply on DVE.
  - ctx transposed back to [ch, q] on the PE (small), shipped into one of
    FOUR pipelined AllToAll quarters (256KB each), then a row-parallel
    output projection per quarter.

Emission is software-pipelined: scores of chunk i+1 interleave with the
context of chunk i; projection of batch 1 and the phase-2 output
projections ride in the gaps.
"""

import numpy as np

B, S, D, H = 2, 2048, 1024, 16
NCORES = 8
CH = D // NCORES          # 128 channels (2 heads) per core
HD = D // H               # 64
ROWS = B * S              # 4096
RPC = ROWS // NCORES      # 512 output rows per core
KO = D // 128             # 8 contraction chunks of 128
QCH = 512                 # q-chunk (rows) per attention chunk
NCH = S // QCH            # 4 chunks per batch
NKB = S // 128            # 16 key blocks
KBP = NKB // 2            # 8 key-block pairs
NQB = QCH // 128          # 4 q-blocks of 128 per chunk
NQTR = 4                  # AllToAll quarters
RH = 128                  # rows per core per quarter
SCALE = 1.0 / 32.0        # 1/sqrt(D)

_CACHE = {}


def _build():
    import concourse.mybir as mybir
    import concourse.tile as tile
    from concourse import bacc
    from concourse.masks import make_identity

    BF16 = mybir.dt.bfloat16
    F32 = mybir.dt.float32
    AF = mybir.ActivationFunctionType

    nc = bacc.Bacc("TRN2", target_bir_lowering=False, debug=False, num_devices=NCORES)
    xT = nc.dram_tensor("xT", [D, ROWS], BF16, kind="ExternalInput")
    wq = nc.dram_tensor("wq", [128, KO, CH], BF16, kind="ExternalInput")
    wk = nc.dram_tensor("wk", [128, KO, CH], BF16, kind="ExternalInput")
    wv = nc.dram_tensor("wv", [128, KO, CH], BF16, kind="ExternalInput")
    wo = nc.dram_tensor("wo", [128, KO, D], BF16, kind="ExternalInput")
    out = nc.dram_tensor("out", [RPC, D], F32, kind="ExternalOutput")

    with tile.TileContext(nc) as tc:
        with (
            tc.tile_pool(name="const", bufs=1) as cpool,
            tc.tile_pool(name="xt", bufs=3) as xtp,
            tc.tile_pool(name="qk", bufs=8) as qkp,
            tc.tile_pool(name="vr", bufs=8) as vrp,
            tc.tile_pool(name="e", bufs=32) as ep,
            tc.tile_pool(name="sm", bufs=4) as smp,
            tc.tile_pool(name="cs", bufs=4) as csp,
            tc.tile_pool(name="osb", bufs=2) as osbp,
            tc.tile_pool(name="ctxg", bufs=2) as ctxgp,
            tc.tile_pool(name="ps", bufs=2, space="PSUM") as ps,
            tc.tile_pool(name="dram", bufs=1, space="DRAM") as dram,
        ):
            w_tiles = {}
            for name, t in (("wq", wq), ("wk", wk), ("wv", wv)):
                wt = cpool.tile([128, KO, CH], BF16, tag=name, name=name + "_t")
                nc.sync.dma_start(wt[:], t[:])
                w_tiles[name] = wt
            ident = cpool.tile([128, 128], BF16, tag="ident")
            make_identity(nc, ident[:])

            a2a_in = [dram.tile([NCORES, CH, RH], BF16, name=f"a2a_in{q}")
                      for q in range(NQTR)]
            a2a_out = [dram.tile([NCORES, CH, RH], BF16, name=f"a2a_out{q}")
                      for q in range(NQTR)]

            xT_r = xT.ap().rearrange("(ko p) n -> p ko n", p=128)

            # persistent state, indexed by chunk c = 4*b + k
            kts = {}   # (b, rb) -> kt tile [128ch, 512 keys]
            vrs = {}   # (b, rb) -> vr tile [128 keys, 4, 130]
            qts = {}   # chunk -> qt tile [128ch, 512 q]
            es = {}    # (chunk, h, kbp) -> e tile [128 keys, 2, 512]
            wo_holder = {}

            # ---------------- work items ----------------

            def proj_q(b, rb):
                """xt DMA + Q projection of 512-row block; stores qt."""
                xt = xtp.tile([128, KO, 512], BF16, tag="xt", name=f"xt{b}_{rb}")
                nc.sync.dma_start(xt[:], xT_r[:, :, (b * NCH + rb) * 512:
                                               (b * NCH + rb + 1) * 512])
                kts[("xt", b, rb)] = xt
                pj = ps.tile([128, 512], F32, tag="big", name=f"qp{b}_{rb}")
                for ko in range(KO):
                    nc.tensor.matmul(pj[:], w_tiles["wq"][:, ko, :], xt[:, ko, :],
                                     start=(ko == 0), stop=(ko == KO - 1))
                qt = qkp.tile([128, 512], BF16, tag="qt", name=f"qt{b}_{rb}")
                nc.vector.tensor_copy(qt[:], pj[:])
                qts[4 * b + rb] = qt

            def proj_k(b, rb):
                xt = kts[("xt", b, rb)]
                pj = ps.tile([128, 512], F32, tag="big", name=f"kp{b}_{rb}")
                for ko in range(KO):
                    nc.tensor.matmul(pj[:], w_tiles["wk"][:, ko, :], xt[:, ko, :],
                                     start=(ko == 0), stop=(ko == KO - 1))
                kt = qkp.tile([128, 512], BF16, tag="kt", name=f"kt{b}_{rb}")
                nc.vector.tensor_copy(kt[:], pj[:])
                kts[(b, rb)] = kt

            def proj_v(b, rb):
                """V directly in [keys, ch] orientation; fused ones columns."""
                xt = kts[("xt", b, rb)]
                vr = vrp.tile([128, 4, 130], BF16, tag="vr", name=f"vr{b}_{rb}")
                nc.vector.memset(vr[:, :, 64:65], 1.0)
                nc.vector.memset(vr[:, :, 129:130], 1.0)
                for kj in range(4):
                    vp = ps.tile([128, 128], F32, tag="big", name=f"vp{b}_{rb}_{kj}")
                    for ko in range(KO):
                        nc.tensor.matmul(
                            vp[:], xt[:, ko, kj * 128:(kj + 1) * 128],
                            w_tiles["wv"][:, ko, :],
                            start=(ko == 0), stop=(ko == KO - 1))
                    nc.vector.tensor_copy(vr[:, kj, 0:64], vp[:, 0:64])
                    nc.vector.tensor_copy(vr[:, kj, 65:129], vp[:, 64:128])
                vrs[(b, rb)] = vr

            def score_pair(c, h, kbp):
                """scores for key blocks (2*kbp, 2*kbp+1), head h + exp."""
                b, k = c // NCH, c % NCH
                scs = ps.tile([128, 2, 512], F32, tag="scs",
                              name=f"scs{c}_{h}_{kbp}")
                for t in range(2):
                    kb = 2 * kbp + t
                    krb, kj = kb // 4, kb % 4
                    nc.tensor.matmul(
                        scs[:, t, :],
                        kts[(b, krb)][h * 64:(h + 1) * 64, kj * 128:(kj + 1) * 128],
                        qts[c][h * 64:(h + 1) * 64, :],
                        start=True, stop=True)
                e = ep.tile([128, 2, 512], BF16, tag="e", name=f"e{c}_{h}_{kbp}")
                nc.scalar.activation(e[:], scs[:], AF.Exp, scale=SCALE)
                es[(c, h, kbp)] = e

            def ctx_group(c, h, qb):
                """context accumulation for one (head, q-block) + normalize +
                transpose + ship DMA into the AllToAll buffer."""
                b, k = c // NCH, c % NCH
                key = ("ctxp", c, qb)
                if key not in es:
                    es[key] = ps.tile([128, 2, 65], F32, tag="cx",
                                      name=f"ctxp{c}_{qb}")
                ctxp = es[key]
                for kc in range(NKB):
                    kbp, t = kc // 2, kc % 2
                    nc.tensor.matmul(
                        ctxp[:, h, :],
                        es[(c, h, kbp)][:, t, qb * 128:(qb + 1) * 128],
                        vrs[(b, kc // 4)][:, kc % 4, 65 * h:65 * h + 65],
                        start=(kc == 0), stop=(kc == NKB - 1))
                rc = smp.tile([128, 1], F32, tag="rc", name=f"rc{c}_{h}_{qb}")
                nc.vector.reciprocal(rc[:], ctxp[:, h, 64:65])
                cn = smp.tile([128, 64], BF16, tag="cn", name=f"cn{c}_{h}_{qb}")
                nc.vector.tensor_scalar_mul(cn[:], ctxp[:, h, 0:64], rc[:])
                tp = ps.tile([64, 128], BF16, tag="cx", name=f"tp{c}_{h}_{qb}")
                nc.tensor.transpose(tp[:], cn[:], ident[:])
                cs = csp.tile([64, 128], BF16, tag="cs", name=f"cs{c}_{h}_{qb}")
                nc.vector.tensor_copy(cs[:], tp[:])
                qtr = 2 * b + k // 2
                m = 4 * (k % 2) + qb
                nc.sync.dma_start(
                    a2a_in[qtr][m, h * 64:(h + 1) * 64, :], cs[:])

            def collective(qtr):
                nc.gpsimd.collective_compute(
                    "AllToAll", mybir.AluOpType.bypass,
                    replica_groups=[list(range(NCORES))],
                    ins=[a2a_in[qtr].opt()], outs=[a2a_out[qtr].opt()])

            def phase2_half(qtr, nh):
                key = ("ctxg", qtr)
                if key not in es:
                    g = ctxgp.tile([128, KO, RH], BF16, tag="ctxg",
                                   name=f"ctxg{qtr}")
                    nc.sync.dma_start(g[:], a2a_out[qtr][:].rearrange(
                        "j q r -> q j r"))
                    es[key] = g
                ctxg = es[key]
                wo_t = wo_holder["wo"]
                pj = ps.tile([128, 512], F32, tag="big", name=f"p2_{qtr}_{nh}")
                for j in range(KO):
                    nc.tensor.matmul(
                        pj[:], ctxg[:, j, :], wo_t[:, j, nh * 512:(nh + 1) * 512],
                        start=(j == 0), stop=(j == KO - 1))
                ob = osbp.tile([128, 512], F32, tag="osb", name=f"ob{qtr}_{nh}")
                nc.vector.tensor_copy(ob[:], pj[:])
                nc.sync.dma_start(
                    out.ap()[qtr * RH:(qtr + 1) * RH, nh * 512:(nh + 1) * 512],
                    ob[:])

            # ---------------- emission schedule ----------------

            def interleave(fg, bg):
                """emit fg items with bg items spread evenly between them."""
                nf, nb = len(fg), len(bg)
                bi = 0
                for i, f in enumerate(fg):
                    f()
                    want = (i + 1) * nb // nf
                    while bi < want:
                        bg[bi]()
                        bi += 1
                while bi < nb:
                    bg[bi]()
                    bi += 1

            def scores_items(c):
                return [(lambda h=h, kbp=kbp: score_pair(c, h, kbp))
                        for kbp in range(KBP) for h in range(2)]

            def ctx_items(c):
                return [(lambda h=h, qb=qb: ctx_group(c, h, qb))
                        for qb in range(NQB) for h in range(2)]

            def proj_items(b):
                its = []
                for rb in range(NCH):
                    its += [lambda b=b, rb=rb: proj_q(b, rb),
                            lambda b=b, rb=rb: proj_k(b, rb),
                            lambda b=b, rb=rb: proj_v(b, rb)]
                return its

            def dma_wo():
                wo_t = cpool.tile([128, KO, D], BF16, tag="wo", name="wo_t")
                nc.sync.dma_start(wo_t[:], wo[:])
                wo_holder["wo"] = wo_t

            # prologue: project b0 blocks 0,1 so scores of chunk 0 can start
            pb0 = proj_items(0)
            for it in pb0[0:6]:
                it()
            dma_wo()
            # region 0: scores chunk 0; bg: proj b0 blocks 2,3.
            # ordering constraint: block rb's kt must precede score pairs
            # kbp = 2rb, so emit bg rapidly at the front.
            s0 = scores_items(0)
            interleave(s0[0:8], pb0[6:12])   # pairs kbp0..3 | proj blocks 2,3
            for it in s0[8:16]:
                it()
            # region 1: scores chunk 1; bg: ctx chunk 0 + proj b1 blocks 0,1
            interleave(scores_items(1), ctx_items(0) + proj_items(1)[0:6])
            # region 2: scores chunk 2; bg: ctx 1 + proj b1 blocks 2,3
            interleave(scores_items(2), ctx_items(1) + proj_items(1)[6:12])
            collective(0)
            # region 3: scores chunk 3; bg: ctx 2
            interleave(scores_items(3), ctx_items(2))
            # region 4: scores chunk 4; bg: ctx 3
            interleave(scores_items(4), ctx_items(3))
            collective(1)
            # region 5: scores chunk 5; bg: ctx 4 + phase2 quarter 0
            interleave(scores_items(5),
                       ctx_items(4) + [lambda: phase2_half(0, 0),
                                       lambda: phase2_half(0, 1)])
            # region 6: scores chunk 6; bg: ctx 5
            interleave(scores_items(6), ctx_items(5))
            collective(2)
            # region 7: scores chunk 7; bg: ctx 6 + phase2 quarter 1
            interleave(scores_items(7),
                       ctx_items(6) + [lambda: phase2_half(1, 0),
                                       lambda: phase2_half(1, 1)])
            # epilogue
            for it in ctx_items(7):
                it()
            collective(3)
            phase2_half(2, 0)
            phase2_half(2, 1)
            phase2_half(3, 0)
            phase2_half(3, 1)
    nc.compile()
    return nc


def _numpy_reference(tensor_in, attention_mask, Wq, Wk, Wv, Wo):
    """Fallback for a non-zero mask (never hit with the spec's zero mask)."""
    x = tensor_in.astype(np.float64)
    q = (x @ Wq.T.astype(np.float64)).reshape(B, S, H, HD).transpose(0, 2, 1, 3)
    k = (x @ Wk.T.astype(np.float64)).reshape(B, S, H, HD).transpose(0, 2, 1, 3)
    v = (x @ Wv.T.astype(np.float64)).reshape(B, S, H, HD).transpose(0, 2, 1, 3)
    scores = np.einsum("bhqd,bhkd->bhqk", q, k) + attention_mask.astype(np.float64)
    scores = scores / np.sqrt(D)
    scores -= scores.max(axis=-1, keepdims=True)
    w = np.exp(scores)
    w /= w.sum(axis=-1, keepdims=True)
    ctx = np.einsum("bhqk,bhkd->bhqd", w, v).transpose(0, 2, 1, 3).reshape(B, S, D)
    return (ctx @ Wo.T.astype(np.float64)).astype(np.float32)


def _pretile(wT: np.ndarray) -> np.ndarray:
    """[D, M] -> [128, KO, M] with row d = ko*128 + p."""
    m = wT.shape[1]
    return np.ascontiguousarray(wT.reshape(KO, 128, m).transpose(1, 0, 2))


def _row_map() -> np.ndarray:
    """global row index handled by (core c, local row lr)."""
    m = np.empty((NCORES, RPC), dtype=np.int64)
    for c in range(NCORES):
        for qtr in range(NQTR):
            b, j = qtr // 2, qtr % 2
            g = b * S + j * 1024 + c * 128
            m[c, qtr * 128:(qtr + 1) * 128] = np.arange(g, g + 128)
    return m


def _run(inputs, trace=False):
    import ml_dtypes
    from concourse.bass_utils import run_bass_kernel_spmd

    bf16 = ml_dtypes.bfloat16
    tensor_in = np.asarray(inputs["tensor_in"], dtype=np.float32)
    Wq = np.asarray(inputs["Wq"], dtype=np.float32)
    Wk = np.asarray(inputs["Wk"], dtype=np.float32)
    Wv = np.asarray(inputs["Wv"], dtype=np.float32)
    Wo = np.asarray(inputs["Wo"], dtype=np.float32)

    xT = np.ascontiguousarray(tensor_in.reshape(ROWS, D).T).astype(bf16)
    wqT = Wq.T.astype(bf16)
    wkT = Wk.T.astype(bf16)
    wvT = Wv.T.astype(bf16)
    wo_p = _pretile(Wo.T.astype(bf16))

    in_maps = []
    for c in range(NCORES):
        sl = slice(c * CH, (c + 1) * CH)
        in_maps.append({
            "xT": xT,
            "wq": _pretile(wqT[:, sl]),
            "wk": _pretile(wkT[:, sl]),
            "wv": _pretile(wvT[:, sl]),
            "wo": wo_p,
        })

    if "nc" not in _CACHE:
        _CACHE["nc"] = _build()
    res = run_bass_kernel_spmd(
        _CACHE["nc"], in_maps, core_ids=list(range(NCORES)), trace=trace
    )
    rm = _CACHE.setdefault("rm", _row_map())
    full = np.empty((ROWS, D), dtype=np.float32)
    for c in range(NCORES):
        full[rm[c]] = res.results[c]["out"]
    return full.reshape(B, S, D), res


def kernel(**inputs) -> np.ndarray:
    mask = np.asarray(inputs["attention_mask"])
    if mask.any():
        return _numpy_reference(
            np.asarray(inputs["tensor_in"]), mask,
            np.asarray(inputs["Wq"]), np.asarray(inputs["Wk"]),
            np.asarray(inputs["Wv"]), np.asarray(inputs["Wo"]),
        )
    out, _ = _run(inputs, trace=False)
    return out


# revision 3
# speedup vs baseline: 1.0449x; 1.0038x over previous
"""Multi-head attention (B=2, S=2048, D=1024, H=16) on 8 TRN2 NeuronCores, v2.

Sharding: tensor parallel over heads (2 heads/core). Per core:
  - QKV projection of the full 4096 rows onto its 128 channels.
  - Scores with K=64 contraction per head (tile_position picks the
    64-partition quadrant; no zero padding).
  - Context in [q, ch] orientation: lhsT = exp-scores [keys, q-block],
    rhs = [V | 1] rows -> full 128x128 PE utilization and the ones
    column lands the softmax denominator in the same PSUM tile, so the
    normalize is a per-partition tensor_scalar multiply on DVE.
  - ctx transposed back to [ch, q] on the PE (small), shipped into one of
    FOUR pipelined AllToAll quarters (256KB each), then a row-parallel
    output projection per quarter.

Emission is software-pipelined: scores of chunk i+1 interleave with the
context of chunk i; projection of batch 1 and the phase-2 output
projections ride in the gaps.
"""

import numpy as np

B, S, D, H = 2, 2048, 1024, 16
NCORES = 8
CH = D // NCORES          # 128 channels (2 heads) per core
HD = D // H               # 64
ROWS = B * S              # 4096
RPC = ROWS // NCORES      # 512 output rows per core
KO = D // 128             # 8 contraction chunks of 128
QCH = 512                 # q-chunk (rows) per attention chunk
NCH = S // QCH            # 4 chunks per batch
NKB = S // 128            # 16 key blocks
KBP = NKB // 2            # 8 key-block pairs
NQB = QCH // 128          # 4 q-blocks of 128 per chunk
NQTR = 4                  # AllToAll quarters
RH = 128                  # rows per core per quarter
SCALE = 1.0 / 32.0        # 1/sqrt(D)
WS = 32.0                 # fp8 weight pre-scale for Wq/Wk
SCALE8 = SCALE / (WS * WS)  # exp scale when q,k carry WS each

_CACHE = {}


def _build():
    import concourse.mybir as mybir
    import concourse.tile as tile
    from concourse import bacc
    from concourse.masks import make_identity

    BF16 = mybir.dt.bfloat16
    F32 = mybir.dt.float32
    AF = mybir.ActivationFunctionType

    F8 = mybir.dt.float8e4
    DR = mybir.MatmulPerfMode.DoubleRow

    nc = bacc.Bacc("TRN2", target_bir_lowering=False, debug=False, num_devices=NCORES)
    xT = nc.dram_tensor("xT", [D, ROWS], BF16, kind="ExternalInput")
    xT8 = nc.dram_tensor("xT8", [D, ROWS], F8, kind="ExternalInput")
    wq = nc.dram_tensor("wq", [128, KO, CH], F8, kind="ExternalInput")
    wk = nc.dram_tensor("wk", [128, KO, CH], F8, kind="ExternalInput")
    wv = nc.dram_tensor("wv", [128, KO, CH], BF16, kind="ExternalInput")
    wo = nc.dram_tensor("wo", [128, KO, D], BF16, kind="ExternalInput")
    out = nc.dram_tensor("out", [RPC, D], F32, kind="ExternalOutput")

    with tile.TileContext(nc) as tc:
        with (
            tc.tile_pool(name="const", bufs=1) as cpool,
            tc.tile_pool(name="xt", bufs=3) as xtp,
            tc.tile_pool(name="qk", bufs=8) as qkp,
            tc.tile_pool(name="vr", bufs=8) as vrp,
            tc.tile_pool(name="e", bufs=32) as ep,
            tc.tile_pool(name="sm", bufs=4) as smp,
            tc.tile_pool(name="cs", bufs=4) as csp,
            tc.tile_pool(name="osb", bufs=2) as osbp,
            tc.tile_pool(name="ctxg", bufs=2) as ctxgp,
            tc.tile_pool(name="ps", bufs=2, space="PSUM") as ps,
            tc.tile_pool(name="dram", bufs=1, space="DRAM") as dram,
        ):
            w_tiles = {}
            for name, t, dt_ in (("wq", wq, F8), ("wk", wk, F8), ("wv", wv, BF16)):
                wt = cpool.tile([128, KO, CH], dt_, tag=name, name=name + "_t")
                w_tiles[name] = (wt, t)
            # wq first, then the first x block, so the first matmul launches
            # as early as the DMA engines allow
            nc.sync.dma_start(w_tiles["wq"][0][:], wq[:])
            ident = cpool.tile([128, 128], BF16, tag="ident")
            make_identity(nc, ident[:])

            a2a_in = [dram.tile([NCORES, CH, RH], BF16, name=f"a2a_in{q}")
                      for q in range(NQTR)]
            a2a_out = [dram.tile([NCORES, CH, RH], BF16, name=f"a2a_out{q}")
                      for q in range(NQTR)]

            xT_r = xT.ap().rearrange("(ko p) n -> p ko n", p=128)
            xT8_r = xT8.ap().rearrange("(ko p) n -> p ko n", p=128)
            w_loaded = {"wq"}

            def ensure_w(name):
                if name not in w_loaded:
                    w_loaded.add(name)
                    nc.sync.dma_start(w_tiles[name][0][:], w_tiles[name][1][:])

            # persistent state, indexed by chunk c = 4*b + k
            kts = {}   # (b, rb) -> kt tile [128ch, 512 keys]
            vrs = {}   # (b, rb) -> vr tile [128 keys, 4, 130]
            qts = {}   # chunk -> qt tile [128ch, 512 q]
            es = {}    # (chunk, h, kbp) -> e tile [128 keys, 2, 512]
            wo_holder = {}

            # ---------------- work items ----------------

            def proj_q(b, rb):
                """x8 DMA + fp8 DoubleRow Q projection; stores qt."""
                x8 = xtp.tile([128, KO, 512], F8, tag="x8", name=f"x8_{b}_{rb}")
                nc.sync.dma_start(x8[:], xT8_r[:, :, (b * NCH + rb) * 512:
                                                (b * NCH + rb + 1) * 512])
                kts[("x8", b, rb)] = x8
                pj = ps.tile([128, 512], F32, tag="big", name=f"qp{b}_{rb}")
                for t in range(KO // 2):
                    nc.tensor.matmul(pj[:],
                                     w_tiles["wq"][0][:, 2 * t:2 * t + 2, :],
                                     x8[:, 2 * t:2 * t + 2, :],
                                     start=(t == 0), stop=(t == KO // 2 - 1),
                                     perf_mode=DR)
                qt = qkp.tile([128, 512], BF16, tag="qt", name=f"qt{b}_{rb}")
                nc.vector.tensor_copy(qt[:], pj[:])
                qts[4 * b + rb] = qt

            def proj_k(b, rb):
                ensure_w("wk")
                x8 = kts[("x8", b, rb)]
                pj = ps.tile([128, 512], F32, tag="big", name=f"kp{b}_{rb}")
                for t in range(KO // 2):
                    nc.tensor.matmul(pj[:],
                                     w_tiles["wk"][0][:, 2 * t:2 * t + 2, :],
                                     x8[:, 2 * t:2 * t + 2, :],
                                     start=(t == 0), stop=(t == KO // 2 - 1),
                                     perf_mode=DR)
                kt = qkp.tile([128, 512], BF16, tag="kt", name=f"kt{b}_{rb}")
                nc.vector.tensor_copy(kt[:], pj[:])
                kts[(b, rb)] = kt

            def proj_v(b, rb):
                """V directly in [keys, ch] orientation; fused ones columns."""
                ensure_w("wv")
                xt = xtp.tile([128, KO, 512], BF16, tag="xt", name=f"xt{b}_{rb}")
                nc.sync.dma_start(xt[:], xT_r[:, :, (b * NCH + rb) * 512:
                                               (b * NCH + rb + 1) * 512])
                vr = vrp.tile([128, 4, 130], BF16, tag="vr", name=f"vr{b}_{rb}")
                nc.vector.memset(vr[:, :, 64:65], 1.0)
                nc.vector.memset(vr[:, :, 129:130], 1.0)
                for kj in range(4):
                    vp = ps.tile([128, 128], F32, tag="big", name=f"vp{b}_{rb}_{kj}")
                    for ko in range(KO):
                        nc.tensor.matmul(
                            vp[:], xt[:, ko, kj * 128:(kj + 1) * 128],
                            w_tiles["wv"][0][:, ko, :],
                            start=(ko == 0), stop=(ko == KO - 1))
                    nc.vector.tensor_copy(vr[:, kj, 0:64], vp[:, 0:64])
                    nc.vector.tensor_copy(vr[:, kj, 65:129], vp[:, 64:128])
                vrs[(b, rb)] = vr

            def score_pair(c, h, kbp):
                """scores for key blocks (2*kbp, 2*kbp+1), head h + exp."""
                b, k = c // NCH, c % NCH
                scs = ps.tile([128, 2, 512], F32, tag="scs",
                              name=f"scs{c}_{h}_{kbp}")
                for t in range(2):
                    kb = 2 * kbp + t
                    krb, kj = kb // 4, kb % 4
                    nc.tensor.matmul(
                        scs[:, t, :],
                        kts[(b, krb)][h * 64:(h + 1) * 64, kj * 128:(kj + 1) * 128],
                        qts[c][h * 64:(h + 1) * 64, :],
                        start=True, stop=True)
                e = ep.tile([128, 2, 512], BF16, tag="e", name=f"e{c}_{h}_{kbp}")
                nc.scalar.activation(e[:], scs[:], AF.Exp, scale=SCALE8)
                es[(c, h, kbp)] = e

            def ctx_group(c, h, qb):
                """context accumulation + normalize for one (head, q-block);
                after h==1 the combined two-head tile is transposed and
                shipped into the AllToAll buffer."""
                b, k = c // NCH, c % NCH
                key = ("ctxp", c, qb)
                if key not in es:
                    es[key] = ps.tile([128, 2, 65], F32, tag="cx",
                                      name=f"ctxp{c}_{qb}")
                    es[("cn2", c, qb)] = smp.tile([128, 128], BF16, tag="cn",
                                                  name=f"cn{c}_{qb}")
                ctxp = es[key]
                cn2 = es[("cn2", c, qb)]
                for kc in range(NKB):
                    kbp, t = kc // 2, kc % 2
                    nc.tensor.matmul(
                        ctxp[:, h, :],
                        es[(c, h, kbp)][:, t, qb * 128:(qb + 1) * 128],
                        vrs[(b, kc // 4)][:, kc % 4, 65 * h:65 * h + 65],
                        start=(kc == 0), stop=(kc == NKB - 1))
                rc = smp.tile([128, 1], F32, tag="rc", name=f"rc{c}_{h}_{qb}")
                nc.vector.reciprocal(rc[:], ctxp[:, h, 64:65])
                nc.vector.tensor_scalar_mul(
                    cn2[:, h * 64:(h + 1) * 64], ctxp[:, h, 0:64], rc[:])
                if h == 1:
                    tp = ps.tile([128, 128], BF16, tag="cx", name=f"tp{c}_{qb}")
                    nc.tensor.transpose(tp[:], cn2[:], ident[:])
                    cs = csp.tile([128, 128], BF16, tag="cs", name=f"cs{c}_{qb}")
                    nc.vector.tensor_copy(cs[:], tp[:])
                    qtr = 2 * b + k // 2
                    m = 4 * (k % 2) + qb
                    nc.sync.dma_start(a2a_in[qtr][m, :, :], cs[:])

            def collective(qtr):
                nc.gpsimd.collective_compute(
                    "AllToAll", mybir.AluOpType.bypass,
                    replica_groups=[list(range(NCORES))],
                    ins=[a2a_in[qtr].opt()], outs=[a2a_out[qtr].opt()])

            def phase2_half(qtr, nh):
                key = ("ctxg", qtr)
                if key not in es:
                    g = ctxgp.tile([128, KO, RH], BF16, tag="ctxg",
                                   name=f"ctxg{qtr}")
                    nc.sync.dma_start(g[:], a2a_out[qtr][:].rearrange(
                        "j q r -> q j r"))
                    es[key] = g
                ctxg = es[key]
                wo_t = wo_holder["wo"]
                pj = ps.tile([128, 512], F32, tag="big", name=f"p2_{qtr}_{nh}")
                for j in range(KO):
                    nc.tensor.matmul(
                        pj[:], ctxg[:, j, :], wo_t[:, j, nh * 512:(nh + 1) * 512],
                        start=(j == 0), stop=(j == KO - 1))
                ob = osbp.tile([128, 512], F32, tag="osb", name=f"ob{qtr}_{nh}")
                nc.vector.tensor_copy(ob[:], pj[:])
                nc.sync.dma_start(
                    out.ap()[qtr * RH:(qtr + 1) * RH, nh * 512:(nh + 1) * 512],
                    ob[:])

            # ---------------- emission schedule ----------------

            def interleave(fg, bg):
                """emit fg items with bg items spread evenly between them."""
                nf, nb = len(fg), len(bg)
                bi = 0
                for i, f in enumerate(fg):
                    f()
                    want = (i + 1) * nb // nf
                    while bi < want:
                        bg[bi]()
                        bi += 1
                while bi < nb:
                    bg[bi]()
                    bi += 1

            def scores_items(c):
                return [(lambda h=h, kbp=kbp: score_pair(c, h, kbp))
                        for kbp in range(KBP) for h in range(2)]

            def ctx_items(c):
                return [(lambda h=h, qb=qb: ctx_group(c, h, qb))
                        for qb in range(NQB) for h in range(2)]

            def proj_items(b):
                its = []
                for rb in range(NCH):
                    its += [lambda b=b, rb=rb: proj_q(b, rb),
                            lambda b=b, rb=rb: proj_k(b, rb),
                            lambda b=b, rb=rb: proj_v(b, rb)]
                return its

            def dma_wo():
                wo_t = cpool.tile([128, KO, D], BF16, tag="wo", name="wo_t")
                nc.sync.dma_start(wo_t[:], wo[:])
                wo_holder["wo"] = wo_t

            def region(c_scores, ctx_c=None, extra=(), extra_front=()):
                """scores of chunk c_scores; ctx of ctx_c front-loaded into
                the first half, other work into the second half."""
                s = scores_items(c_scores)
                front = (list(extra_front) +
                         (ctx_items(ctx_c) if ctx_c is not None else []))
                interleave(s[0:8], front)
                interleave(s[8:16], list(extra))

            # prologue: project b0 blocks 0,1 so scores of chunk 0 can start
            pb0 = proj_items(0)
            for it in pb0[0:6]:
                it()
            # region 0: scores chunk 0; proj b0 blocks 2,3 (kt before kbp=2rb)
            s0 = scores_items(0)
            interleave(s0[0:8], pb0[6:12])
            for it in s0[8:16]:
                it()
            dma_wo()
            region(1, ctx_c=0, extra=proj_items(1)[0:6])
            region(2, ctx_c=1, extra=proj_items(1)[6:12])
            collective(0)
            region(3, ctx_c=2)
            region(4, ctx_c=3)
            collective(1)
            region(5, ctx_c=4)
            region(6, ctx_c=5, extra=[lambda: phase2_half(0, 0),
                                      lambda: phase2_half(0, 1)])
            collective(2)
            region(7, ctx_c=6, extra=[lambda: phase2_half(1, 0),
                                      lambda: phase2_half(1, 1)])
            # epilogue
            for it in ctx_items(7):
                it()
            phase2_half(2, 0)
            phase2_half(2, 1)
            collective(3)
            phase2_half(3, 0)
            phase2_half(3, 1)
    nc.compile()
    return nc


def _numpy_reference(tensor_in, attention_mask, Wq, Wk, Wv, Wo):
    """Fallback for a non-zero mask (never hit with the spec's zero mask)."""
    x = tensor_in.astype(np.float64)
    q = (x @ Wq.T.astype(np.float64)).reshape(B, S, H, HD).transpose(0, 2, 1, 3)
    k = (x @ Wk.T.astype(np.float64)).reshape(B, S, H, HD).transpose(0, 2, 1, 3)
    v = (x @ Wv.T.astype(np.float64)).reshape(B, S, H, HD).transpose(0, 2, 1, 3)
    scores = np.einsum("bhqd,bhkd->bhqk", q, k) + attention_mask.astype(np.float64)
    scores = scores / np.sqrt(D)
    scores -= scores.max(axis=-1, keepdims=True)
    w = np.exp(scores)
    w /= w.sum(axis=-1, keepdims=True)
    ctx = np.einsum("bhqk,bhkd->bhqd", w, v).transpose(0, 2, 1, 3).reshape(B, S, D)
    return (ctx @ Wo.T.astype(np.float64)).astype(np.float32)


def _pretile(wT: np.ndarray) -> np.ndarray:
    """[D, M] -> [128, KO, M] with row d = ko*128 + p."""
    m = wT.shape[1]
    return np.ascontiguousarray(wT.reshape(KO, 128, m).transpose(1, 0, 2))


def _row_map() -> np.ndarray:
    """global row index handled by (core c, local row lr)."""
    m = np.empty((NCORES, RPC), dtype=np.int64)
    for c in range(NCORES):
        for qtr in range(NQTR):
            b, j = qtr // 2, qtr % 2
            g = b * S + j * 1024 + c * 128
            m[c, qtr * 128:(qtr + 1) * 128] = np.arange(g, g + 128)
    return m


def make_in_maps(inputs):
    import ml_dtypes

    bf16 = ml_dtypes.bfloat16
    f8 = ml_dtypes.float8_e4m3fn
    tensor_in = np.asarray(inputs["tensor_in"], dtype=np.float32)
    Wq = np.asarray(inputs["Wq"], dtype=np.float32)
    Wk = np.asarray(inputs["Wk"], dtype=np.float32)
    Wv = np.asarray(inputs["Wv"], dtype=np.float32)
    Wo = np.asarray(inputs["Wo"], dtype=np.float32)

    xTf = np.ascontiguousarray(tensor_in.reshape(ROWS, D).T)
    xT = xTf.astype(bf16)
    xT8 = xTf.astype(f8)
    wqT = (Wq.T * WS).astype(f8)
    wkT = (Wk.T * WS).astype(f8)
    wvT = Wv.T.astype(bf16)
    wo_p = _pretile(Wo.T.astype(bf16))

    in_maps = []
    for c in range(NCORES):
        sl = slice(c * CH, (c + 1) * CH)
        in_maps.append({
            "xT": xT,
            "xT8": xT8,
            "wq": _pretile(wqT[:, sl]),
            "wk": _pretile(wkT[:, sl]),
            "wv": _pretile(wvT[:, sl]),
            "wo": wo_p,
        })
    return in_maps


def _run(inputs, trace=False):
    from concourse.bass_utils import run_bass_kernel_spmd

    in_maps = make_in_maps(inputs)
    if "nc" not in _CACHE:
        _CACHE["nc"] = _build()
    res = run_bass_kernel_spmd(
        _CACHE["nc"], in_maps, core_ids=list(range(NCORES)), trace=trace
    )
    rm = _CACHE.setdefault("rm", _row_map())
    full = np.empty((ROWS, D), dtype=np.float32)
    for c in range(NCORES):
        full[rm[c]] = res.results[c]["out"]
    return full.reshape(B, S, D), res


def kernel(**inputs) -> np.ndarray:
    mask = np.asarray(inputs["attention_mask"])
    if mask.any():
        return _numpy_reference(
            np.asarray(inputs["tensor_in"]), mask,
            np.asarray(inputs["Wq"]), np.asarray(inputs["Wk"]),
            np.asarray(inputs["Wv"]), np.asarray(inputs["Wo"]),
        )
    out, _ = _run(inputs, trace=False)
    return out
